# revision 1
# baseline (speedup 1.0000x reference)
"""CACombiner Trainium2 kernel: conv-projected efficient attention + FFN.

Data-parallel over batch: 8 batch elements -> 8 NeuronCores, identical SPMD
program per core. All heavy matmuls run as float32r (full PE rate); the
attention-weight path (exp(k), v, softmax(q), ctx) runs in bf16.
"""
import sys
sys.path.insert(0, "/opt/trn_rl_repo")
from contextlib import ExitStack

import numpy as np

import concourse.bass as bass
import concourse.tile as tile
from concourse import mybir, bacc
from concourse.bass_utils import run_bass_kernel_spmd
from concourse.alu_op_type import AluOpType

F32 = mybir.dt.float32
F32R = mybir.dt.float32r
BF16 = mybir.dt.bfloat16
AFT = mybir.ActivationFunctionType
Ax = mybir.AxisListType

B, C, L = 8, 512, 4096
H, DK = 8, 64
EPS = 1e-5
CC = C // 128          # 4 channel chunks
NL1 = L // 128         # 32 phase-1 l-tiles
NL2 = L // 512         # 8 phase-2 l-tiles

_CACHE = {}
LAST_RESULT = None


def _build_program():
    nc = bacc.Bacc("TRN2", target_bir_lowering=False, debug=False)

    def din(name, shape, dtype):
        return nc.dram_tensor(name, list(shape), dtype, kind="ExternalInput").ap()

    z1d = din("z1", (C, L), F32R)
    z2d = din("z2", (C, L), F32R)
    WqTt_d = din("WqTt", (128, CC, 512), F32R)
    bq_row_d = din("bq_row", (1, 512), F32R)
    WkvTt_d = din("WkvTt", (128, CC, 1024), F32R)
    WrTt_d = din("WrTt", (128, CC, 512), F32R)
    W1gTt_d = din("W1gTt", (128, CC, 1024), F32R)
    W2gTt_d = din("W2gTt", (128, 8, 512), F32R)
    U1W_d = din("U1W", (2, 1024), F32R)
    u2ct_d = din("u2ct", (128, 8), F32R)
    G2B_d = din("G2B", (2, 512), F32R)
    ivgt_d = din("ivgt", (128, CC), F32R)
    inv512_d = din("inv512", (128, 1), F32R)
    ones1x128_d = din("ones1x128", (1, 128), F32R)
    ident_d = din("ident", (128, 128), BF16)
    br_c_d = din("br_c", (128, CC), F32)
    bv_c_d = din("bv_c", (128, CC), F32)
    be2_c_d = din("be2_c", (128, CC), F32)
    eps_c_d = din("eps_c", (128, 1), F32)
    ones_row_d = din("ones_row", (1, 512), F32R)
    outd = nc.dram_tensor("out", [C, L], F32, kind="ExternalOutput").ap()

    z1r = z1d.rearrange("(cc p) l -> p cc l", p=128)
    z2r = z2d.rearrange("(cc p) l -> p cc l", p=128)

    mm = nc.tensor.matmul
    tt = nc.vector.tensor_tensor
    ts = nc.vector.tensor_scalar
    stt = nc.vector.scalar_tensor_tensor
    act = nc.scalar.activation

    with tile.TileContext(nc) as tc, ExitStack() as ctx:
        cpool = ctx.enter_context(tc.tile_pool(name="consts", bufs=1))

        def const_tile(shape, dtype, src, tag):
            t = cpool.tile(list(shape), dtype, tag=tag, name=tag)
            nc.sync.dma_start(t[:], src)
            return t

        WqTt = const_tile((128, CC, 512), F32R, WqTt_d, "WqTt")
        bq_row = const_tile((1, 512), F32R, bq_row_d, "bq_row")
        WkvTt = const_tile((128, CC, 1024), F32R, WkvTt_d, "WkvTt")
        WrTt = const_tile((128, CC, 512), F32R, WrTt_d, "WrTt")
        W1gTt = const_tile((128, CC, 1024), F32R, W1gTt_d, "W1gTt")
        W2gTt = const_tile((128, 8, 512), F32R, W2gTt_d, "W2gTt")
        U1W = const_tile((2, 1024), F32R, U1W_d, "U1W")
        u2ct = const_tile((128, 8), F32R, u2ct_d, "u2ct")
        G2B = const_tile((2, 512), F32R, G2B_d, "G2B")
        ivgt = const_tile((128, CC), F32R, ivgt_d, "ivgt")
        inv512 = const_tile((128, 1), F32R, inv512_d, "inv512")
        ones1x128 = const_tile((1, 128), F32R, ones1x128_d, "ones1x128")
        ident = const_tile((128, 128), BF16, ident_d, "ident")
        br_c = const_tile((128, CC), F32, br_c_d, "br_c")
        bv_c = const_tile((128, CC), F32, bv_c_d, "bv_c")
        be2_c = const_tile((128, CC), F32, be2_c_d, "be2_c")
        eps_c = const_tile((128, 1), F32, eps_c_d, "eps_c")
        ones_row = const_tile((1, 512), F32R, ones_row_d, "ones_row")

        # persistent across phases
        qsm = cpool.tile([128, CC, L], BF16, tag="qsm", name="qsm")      # softmaxed q, channels-first
        ctxbd = [cpool.tile([128, 128], BF16, tag=f"ctxbd{p}", name=f"ctxbd{p}") for p in range(CC)]

        # ---------------- Phase 1: q softmax + k/v + ctx accumulation ----------------
        with ExitStack() as p1:
            lp1 = p1.enter_context(tc.tile_pool(name="lp1", bufs=2))
            ps_ctx = p1.enter_context(tc.tile_pool(name="ps_ctx", bufs=1, space="PSUM"))
            ps_w = p1.enter_context(tc.tile_pool(name="ps_w", bufs=1, space="PSUM"))

            ctxps = [ps_ctx.tile([128, 129], F32, tag=f"ctx{p}", name=f"ctxps{p}") for p in range(CC)]

            for lt in range(NL1):
                sl = slice(lt * 128, (lt + 1) * 128)
                z1c = lp1.tile([128, CC, 128], F32R, tag="z1c")
                nc.sync.dma_start(z1c[:], z1r[:, :, sl])
                z2c = lp1.tile([128, CC, 128], F32R, tag="z2c")
                nc.sync.dma_start(z2c[:], z2r[:, :, sl])

                # qT [l,128][o,512] = z1^T Wq^T + bq
                qps = ps_w.tile([128, 512], F32, tag="qps")
                for cc in range(CC):
                    mm(qps[:], z1c[:, cc, :], WqTt[:, cc, :], start=(cc == 0), stop=False)
                mm(qps[:], ones1x128[:], bq_row[:], start=False, stop=True)

                # exp + per-head sums (ACT accumulate), then normalize
                EqT = lp1.tile([128, 512], F32, tag="EqT")
                Sq = lp1.tile([128, 8], F32, tag="Sq")
                for h in range(H):
                    hs = slice(h * 64, (h + 1) * 64)
                    act(EqT[:, hs], qps[:, hs], AFT.Exp, accum_out=Sq[:, h:h + 1])
                rq = lp1.tile([128, 8], F32, tag="rq")
                nc.vector.reciprocal(rq[:], Sq[:])
                qsmT = lp1.tile([128, 512], BF16, tag="qsmT")
                tt(qsmT[:].rearrange("p (g x) -> p g x", x=64),
                   EqT[:].rearrange("p (g x) -> p g x", x=64),
                   rq[:].unsqueeze(2).broadcast_to([128, 8, 64]), AluOpType.mult)

                # transpose qsmT back to channels-first into qsm
                tps = ps_w.tile([128, 512], BF16, tag="tps")
                for cc in range(CC):
                    cs = slice(cc * 128, (cc + 1) * 128)
                    nc.tensor.transpose(tps[:, cs], qsmT[:, cs], ident[:])
                nc.vector.tensor_copy(
                    qsm[:, :, sl],
                    tps[:].rearrange("p (cc x) -> p cc x", x=128))

                # kT | vT
                kvps = ps_w.tile([128, 1024], F32, tag="kvps")
                for cc in range(CC):
                    mm(kvps[:, 0:512], z2c[:, cc, :], WkvTt[:, cc, 0:512],
                       start=(cc == 0), stop=(cc == CC - 1))
                for cc in range(CC):
                    mm(kvps[:, 512:1024], z2c[:, cc, :], WkvTt[:, cc, 512:1024],
                       start=(cc == 0), stop=(cc == CC - 1))
                EkT = lp1.tile([128, 512], BF16, tag="EkT")
                act(EkT[:], kvps[:, 0:512], AFT.Exp)
                vT = lp1.tile([128, 516], BF16, tag="vT")
                nc.vector.tensor_copy(
                    vT[:].rearrange("p (pr x) -> p pr x", pr=4)[:, :, 0:128],
                    kvps[:, 512:1024].rearrange("p (pr x) -> p pr x", pr=4))
                nc.vector.memset(vT[:].rearrange("p (pr x) -> p pr x", pr=4)[:, :, 128:129], 1.0)

                # ctx accumulation: per head-pair [2heads-k, 2heads-v | S]
                for pr in range(CC):
                    mm(ctxps[pr][:], EkT[:, pr * 128:(pr + 1) * 128],
                       vT[:, pr * 129:(pr + 1) * 129],
                       start=(lt == 0), stop=(lt == NL1 - 1), skip_group_check=True)

            # finalize ctx: normalize rows by S, build block-diagonal bf16 tiles
            for pr in range(CC):
                rs = lp1.tile([128, 1], F32, tag="rs")
                nc.vector.reciprocal(rs[:], ctxps[pr][:, 128:129])
                nc.vector.memset(ctxbd[pr][:], 0.0)
                ts(ctxbd[pr][0:64, 0:64], ctxps[pr][0:64, 0:64], rs[0:64, :], None,
                   AluOpType.mult)
                ts(ctxbd[pr][64:128, 64:128], ctxps[pr][64:128, 64:128], rs[64:128, :], None,
                   AluOpType.mult)

        # ---------------- Phase 2: attention apply + reprojection + LN/FFN ----------------
        with ExitStack() as p2:
            lp2 = p2.enter_context(tc.tile_pool(name="lp2", bufs=2))
            lph = p2.enter_context(tc.tile_pool(name="lph", bufs=1))
            ps_big = p2.enter_context(tc.tile_pool(name="ps_big", bufs=5, space="PSUM"))
            ps_row = p2.enter_context(tc.tile_pool(name="ps_row", bufs=2, space="PSUM"))

            for lt in range(NL2):
                sl = slice(lt * 512, (lt + 1) * 512)
                z1res = lp2.tile([128, CC, 512], F32R, tag="z1res", bufs=1)
                nc.sync.dma_start(z1res[:], z1r[:, :, sl])

                # att[v,l] = ctx_bd @ qsm + bv
                att = []
                for pr in range(CC):
                    aps = ps_big.tile([128, 512], F32, tag="big")
                    mm(aps[:], ctxbd[pr][:], qsm[:, pr, sl], start=True, stop=True)
                    a = lph.tile([128, 512], F32R, tag=f"att{pr}")
                    ts(a[:], aps[:], bv_c[:, pr:pr + 1], None, AluOpType.add)
                    att.append(a)

                # z = Wr att + br + z1
                zt = []
                for ot in range(CC):
                    zps = ps_big.tile([128, 512], F32, tag="big")
                    for pr in range(CC):
                        mm(zps[:], WrTt[:, pr, ot * 128:(ot + 1) * 128], att[pr][:],
                           start=(pr == 0), stop=(pr == CC - 1))
                    z = lph.tile([128, 512], F32R, tag=f"z{ot}")
                    stt(z[:], zps[:], br_c[:, ot:ot + 1], z1res[:, ot, :].bitcast(F32),
                        AluOpType.add, AluOpType.add)
                    zt.append(z)

                # LN1 stats rows
                mups = ps_row.tile([1, 512], F32, tag="row")
                for ot in range(CC):
                    mm(mups[:], inv512[:], zt[ot][:], start=(ot == 0), stop=(ot == CC - 1))
                e2ps = ps_row.tile([1, 512], F32, tag="row")
                for ot in range(CC):
                    zsq = lp2.tile([128, 512], F32R, tag="zsq")
                    act(zsq[:], zt[ot][:].bitcast(F32), AFT.Square)
                    mm(e2ps[:], inv512[:], zsq[:], start=(ot == 0), stop=(ot == CC - 1))
                murow = lp2.tile([1, 512], F32, tag="murow", bufs=1)
                nc.vector.tensor_copy(murow[:], mups[:])
                musq = lp2.tile([1, 512], F32, tag="musq", bufs=1)
                tt(musq[:], murow[:], murow[:], AluOpType.mult)
                varrow = lp2.tile([1, 512], F32, tag="varrow", bufs=1)
                tt(varrow[:], e2ps[:], musq[:], AluOpType.subtract)
                sig = lp2.tile([1, 512], F32, tag="sig", bufs=1)
                act(sig[:], varrow[:], AFT.Sqrt, bias=eps_c[0:1, :])
                rhs2 = lp2.tile([2, 512], F32R, tag="rhs2", bufs=1)
                ts(rhs2[0:1, :], mups[:], -1.0, None, AluOpType.mult)
                sigR = lp2.tile([1, 512], F32R, tag="sigR", bufs=1)
                nc.vector.tensor_copy(sigR[:], sig[:])
                nc.sync.dma_start(rhs2[1:2, :], sigR[:])
                invsF = lp2.tile([1, 512], F32, tag="invsF", bufs=1)
                nc.vector.reciprocal(invsF[:], sig[:])
                invs = lp2.tile([1, 512], F32R, tag="invs", bufs=1)
                nc.vector.tensor_copy(invs[:], invsF[:])
                bc = ps_big.tile([128, 512], F32, tag="big")
                mm(bc[:], ones1x128[:], invs[:], start=True, stop=True)
                invsb = lp2.tile([128, 512], F32, tag="invsb", bufs=1)
                nc.vector.tensor_copy(invsb[:], bc[:])

                # FFN1 + ELU + FFN2 accumulation (j-outer so hE slots rotate)
                f2ps = [ps_big.tile([128, 512], F32, tag="big", name=f"f2ps{o2}")
                        for o2 in range(CC)]
                mu2 = ps_row.tile([1, 512], F32, tag="row", name="mu2")
                for j in range(8):
                    fps = ps_big.tile([128, 512], F32, tag="big", name="fps")
                    for cc in range(CC):
                        mm(fps[:], W1gTt[:, cc, j * 128:(j + 1) * 128], zt[cc][:],
                           start=(cc == 0), stop=False)
                    mm(fps[:], U1W[:, j * 128:(j + 1) * 128], rhs2[:], start=False, stop=True)
                    hp = lp2.tile([128, 512], F32, tag="hp")
                    tt(hp[:], fps[:], invsb[:], AluOpType.mult)
                    E = lp2.tile([128, 512], F32, tag="E")
                    act(E[:], hp[:], AFT.Exp)
                    nc.gpsimd.tensor_scalar(E[:], E[:], 1.0, -1.0, AluOpType.min,
                                            AluOpType.add)
                    he = lph.tile([128, 512], F32R, tag="hE", bufs=3, name="he")
                    stt(he[:], hp[:], 0.0, E[:], AluOpType.max, AluOpType.add)
                    for o2 in range(CC):
                        mm(f2ps[o2][:], W2gTt[:, j, o2 * 128:(o2 + 1) * 128], he[:],
                           start=(j == 0), stop=False, skip_group_check=True)
                    mm(mu2[:], u2ct[:, j:j + 1], he[:], start=(j == 0), stop=(j == 7),
                       skip_group_check=True)
                rhs2b = lp2.tile([2, 512], F32R, tag="rhs2b", bufs=1)
                nc.sync.dma_start(rhs2b[0:1, :], ones_row[:])
                negmu2 = lp2.tile([1, 512], F32R, tag="negmu2", bufs=1)
                ts(negmu2[:], mu2[:], -1.0, B2MEAN_PLACEHOLDER, AluOpType.mult,
                   AluOpType.subtract)
                nc.sync.dma_start(rhs2b[1:2, :], negmu2[:])
                yg = []
                for o2 in range(CC):
                    mm(f2ps[o2][:], G2B[:, o2 * 128:(o2 + 1) * 128], rhs2b[:],
                       start=False, stop=True, skip_group_check=True)
                    y = lph.tile([128, 512], F32, tag=f"yg{o2}", name=f"yg{o2}")
                    nc.vector.tensor_copy(y[:], f2ps[o2][:])
                    yg.append(y)

                # LN2 variance + apply
                v2ps = ps_row.tile([1, 512], F32, tag="row")
                for o2 in range(CC):
                    sq2 = lp2.tile([128, 512], F32R, tag="sq2")
                    act(sq2[:], yg[o2][:], AFT.Square)
                    mm(v2ps[:], ivgt[:, o2:o2 + 1], sq2[:], start=(o2 == 0),
                       stop=(o2 == CC - 1))
                sig2 = lp2.tile([1, 512], F32, tag="sig2", bufs=1)
                act(sig2[:], v2ps[:], AFT.Sqrt, bias=eps_c[0:1, :])
                invs2F = lp2.tile([1, 512], F32, tag="invs2F", bufs=1)
                nc.vector.reciprocal(invs2F[:], sig2[:])
                invs2 = lp2.tile([1, 512], F32R, tag="invs2", bufs=1)
                nc.vector.tensor_copy(invs2[:], invs2F[:])
                bc2 = ps_big.tile([128, 512], F32, tag="big")
                mm(bc2[:], ones1x128[:], invs2[:], start=True, stop=True)
                invsb2 = lp2.tile([128, 512], F32, tag="invsb2", bufs=1)
                nc.vector.tensor_copy(invsb2[:], bc2[:])
                for o2 in range(CC):
                    tt(yg[o2][:], yg[o2][:], invsb2[:], AluOpType.mult)
                    ot_t = lp2.tile([128, 512], F32, tag="ot")
                    nc.gpsimd.tensor_scalar(ot_t[:], yg[o2][:], be2_c[:, o2:o2 + 1],
                                            None, AluOpType.add)
                    nc.sync.dma_start(outd[o2 * 128:(o2 + 1) * 128, sl], ot_t[:])

    nc.compile()
    return nc


def _prep_consts(Wq, bq, Wk, bk, Wv, bv, Wr, br, g1, be1, W1, b1, W2, b2, g2, be2):
    f = np.float32
    WqT = np.ascontiguousarray(Wq.T, dtype=f)                       # [c, o]
    WkvT = np.concatenate([Wk.T, Wv.T], axis=1).astype(f)           # [c, k|v]
    WrT = np.ascontiguousarray(Wr.T, dtype=f)                       # [v, o]
    W1g = (W1 * g1[None, :]).astype(f)                              # [1024, c]
    W1gT = np.ascontiguousarray(W1g.T)                              # [c, 1024]
    W2g = (W2 * g2[:, None]).astype(f)                              # [c, 1024h]
    W2gT = np.ascontiguousarray(W2g.T)                              # [h, c]
    u1 = W1g.sum(axis=1).astype(f)
    w1bb = (W1 @ be1 + b1).astype(f)
    u2 = (W2.sum(axis=0) / 512.0).astype(f)
    ivg = (1.0 / (512.0 * g2 * g2)).astype(f)
    b2mean = float(np.mean(b2))

    def chunkT(a, n):          # [n*128, m] -> [128, n, m]
        return np.ascontiguousarray(a.reshape(n, 128, -1).transpose(1, 0, 2))

    def colsT(v, n):           # [n*128] -> [128, n]
        return np.ascontiguousarray(v.reshape(n, 128).T)

    return {
        "WqTt": chunkT(WqT, CC),
        "bq_row": bq.reshape(1, 512).astype(f),
        "WkvTt": chunkT(WkvT, CC),
        "WrTt": chunkT(WrT, CC),
        "W1gTt": chunkT(W1gT, CC),
        "W2gTt": chunkT(W2gT, 8),
        "U1W": np.stack([u1, w1bb]).astype(f),
        "u2ct": colsT(u2, 8),
        "G2B": np.stack([(g2 * b2).astype(f), g2.astype(f)]),
        "ivgt": colsT(ivg, CC),
        "inv512": np.full((128, 1), 1.0 / 512.0, dtype=f),
        "ones1x128": np.ones((1, 128), dtype=f),
        "ident": np.eye(128, dtype=f).astype(np.dtype("bfloat16") if False else f),
        "br_c": colsT(br.astype(f), CC),
        "bv_c": colsT(bv.astype(f), CC),
        "be2_c": colsT(be2.astype(f), CC),
        "eps_c": np.full((128, 1), EPS, dtype=f),
        "ones_row": np.ones((1, 512), dtype=f),
    }, b2mean


def kernel(**inputs):
    global LAST_RESULT
    import ml_dtypes
    z1 = np.asarray(inputs["z1"], dtype=np.float32)
    z2 = np.asarray(inputs["z2"], dtype=np.float32)
    consts, b2mean = _prep_consts(
        *[np.asarray(inputs[k], dtype=np.float32) for k in
          ["Wq", "bq", "Wk", "bk", "Wv", "bv", "Wr", "br", "g1", "be1",
           "W1", "b1", "W2", "b2", "g2", "be2"]])
    consts["ident"] = np.eye(128, dtype=ml_dtypes.bfloat16)

    key = ("prog", round(b2mean * 1e9))
    if key not in _CACHE:
        global B2MEAN_PLACEHOLDER
        B2MEAN_PLACEHOLDER = b2mean
        _CACHE.clear()
        _CACHE[key] = _build_program()
    nc = _CACHE[key]

    in_maps = []
    for b in range(B):
        m = dict(consts)
        m["z1"] = np.ascontiguousarray(z1[b])
        m["z2"] = np.ascontiguousarray(z2[b])
        in_maps.append(m)

    import os
    trace = bool(int(os.environ.get("KERNEL_TRACE", "0")))
    res = run_bass_kernel_spmd(nc, in_maps, list(range(B)), trace=trace)
    LAST_RESULT = res
    out = np.stack([res.results[b]["out"] for b in range(B)], axis=0)
    return out.astype(np.float32)


B2MEAN_PLACEHOLDER = 0.0



# revision 11
# speedup vs baseline: 1.3398x; 1.3398x over previous
"""CACombiner Trainium2 kernel: conv-projected efficient attention + FFN.

Data-parallel over batch: 8 batch elements -> 8 NeuronCores, identical SPMD
program per core.

v2: the attention path (q/k/v projections, ctx accumulation, reprojection)
runs in fp8e4m3 with DoubleRow matmuls (K=256 per instruction at 0.5
cyc/row).  The attention branch contributes ~0.3% of the residual stream, so
fp8 noise there is negligible.  Biases are folded exactly:
  - bk cancels in the key softmax (constant along L per channel);
  - bv folds into the normalized ctx (+bv[v] per column);
  - bq folds as e^{bq} row weights into ctx2 and the deferred softmax-q
    normalization sum.
Softmax-q normalization is deferred to phase 2 (unnormalized exp(q) kept
channels-first in bf16; per-(head,token) 1/sum applied after the ctx apply).
LayerNorm rsqrt = exp(-0.5*ln(var+eps)) keeps every activation in one act
table set (no table reloads).  ELU = min(e^x - 1, max(x, 0)).
FFN matmuls keep full-precision f32r weights with bf16 moving operands.
"""
import sys
sys.path.insert(0, "/opt/trn_rl_repo")
from contextlib import ExitStack

import numpy as np

import concourse.bass as bass
import concourse.tile as tile
from concourse import mybir, bacc
from concourse.bass_utils import run_bass_kernel_spmd
from concourse.alu_op_type import AluOpType

F32 = mybir.dt.float32
F32R = mybir.dt.float32r
BF16 = mybir.dt.bfloat16
FP8 = mybir.dt.float8e4
AFT = mybir.ActivationFunctionType
DR = mybir.MatmulPerfMode.DoubleRow

B, C, L = 8, 512, 4096
H, DK = 8, 64
EPS = 1e-5
CC = C // 128           # 4 channel chunks
NL1 = L // 128          # 32 phase-1 l-tiles
NL2 = L // 512          # 8 phase-2 l-tiles
WS = 32.0               # fp8 weight scale for Wq/Wk/Wv/Wr
AS = 64.0               # att fp8 scale
SZ = 1.0 / (WS * AS)    # undo both scales after the Wr matmul

_CACHE = {}
LAST_RESULT = None


def _build_program():
    nc = bacc.Bacc("TRN2", target_bir_lowering=False, debug=False)

    def din(name, shape, dtype):
        return nc.dram_tensor(name, list(shape), dtype, kind="ExternalInput").ap()

    z1d = din("z1", (C, L), F32R)
    z2d = din("z2", (C, L), F32R)
    Wq8T_d = din("Wq8T", (128, CC, 512), FP8)
    Wk8T_d = din("Wk8T", (128, CC, 512), FP8)
    Wv8T_d = din("Wv8T", (128, CC, 512), FP8)
    Wr8T_d = din("Wr8T", (128, CC, 512), FP8)
    W1gTt_d = din("W1gTt", (128, CC, 1024), F32R)
    W2gTt_d = din("W2gTt", (128, 8, 512), BF16)
    u1neg_d = din("u1neg", (1, 1024), F32R)
    w1bbc_d = din("w1bbc", (128, 8), F32)
    u2ct_d = din("u2ct", (128, 8), BF16)
    g2b2row_d = din("g2b2row", (1, 512), F32R)
    g2row_d = din("g2row", (1, 512), F32R)
    ones_row_d = din("ones_row", (1, 512), F32R)
    ivgt_d = din("ivgt", (128, CC), F32R)
    inv512_d = din("inv512", (128, 1), F32R)
    ones1x128_d = din("ones1x128", (1, 128), F32R)
    ident_d = din("ident", (128, 128), BF16)
    br_c_d = din("br_c", (128, CC), F32)
    be2_c_d = din("be2_c", (128, CC), F32)
    eps_c_d = din("eps_c", (128, 1), F32)
    brm_c_d = din("brm_c", (128, 1), F32)
    negb2m_c_d = din("negb2m_c", (128, 1), F32)
    ebqH_d = din("ebqH", (128, CC, 8), BF16)
    maskH64_d = din("maskH64", (8, CC, 128), F32R)
    bvqbd_d = din("bvqbd", (128, CC, 128), F32)
    ebqcol_d = din("ebqcol", (128, CC), F32)
    outd = nc.dram_tensor("out", [C, L], F32, kind="ExternalOutput").ap()

    z1r = z1d.rearrange("(cc p) l -> p cc l", p=128)
    z2r = z2d.rearrange("(cc p) l -> p cc l", p=128)

    mm = nc.tensor.matmul
    tt = nc.vector.tensor_tensor
    ts = nc.vector.tensor_scalar
    stt = nc.vector.scalar_tensor_tensor
    ptt = nc.gpsimd.tensor_tensor
    pts = nc.gpsimd.tensor_scalar
    pstt = nc.gpsimd.scalar_tensor_tensor
    act = nc.scalar.activation

    with tile.TileContext(nc) as tc, ExitStack() as ctx:
        cpool = ctx.enter_context(tc.tile_pool(name="consts", bufs=1))

        def const_tile(shape, dtype, src, tag):
            t = cpool.tile(list(shape), dtype, tag=tag, name=tag)
            nc.sync.dma_start(t[:], src)
            return t

        Wq8T = const_tile((128, CC, 512), FP8, Wq8T_d, "Wq8T")
        Wk8T = const_tile((128, CC, 512), FP8, Wk8T_d, "Wk8T")
        Wv8T = const_tile((128, CC, 512), FP8, Wv8T_d, "Wv8T")
        Wr8T = const_tile((128, CC, 512), FP8, Wr8T_d, "Wr8T")
        W1gTt = const_tile((128, CC, 1024), F32R, W1gTt_d, "W1gTt")
        W2gTt = const_tile((128, 8, 512), BF16, W2gTt_d, "W2gTt")
        u1neg = const_tile((1, 1024), F32R, u1neg_d, "u1neg")
        w1bbc = const_tile((128, 8), F32, w1bbc_d, "w1bbc")
        u2ct = const_tile((128, 8), BF16, u2ct_d, "u2ct")
        g2b2row = const_tile((1, 512), F32R, g2b2row_d, "g2b2row")
        g2row = const_tile((1, 512), F32R, g2row_d, "g2row")
        ones_row = const_tile((1, 512), F32R, ones_row_d, "ones_row")
        ivgt = const_tile((128, CC), F32R, ivgt_d, "ivgt")
        inv512 = const_tile((128, 1), F32R, inv512_d, "inv512")
        ones1x128 = const_tile((1, 128), F32R, ones1x128_d, "ones1x128")
        ident = const_tile((128, 128), BF16, ident_d, "ident")
        br_c = const_tile((128, CC), F32, br_c_d, "br_c")
        be2_c = const_tile((128, CC), F32, be2_c_d, "be2_c")
        eps_c = const_tile((128, 1), F32, eps_c_d, "eps_c")
        brm_c = const_tile((128, 1), F32, brm_c_d, "brm_c")
        negb2m_c = const_tile((128, 1), F32, negb2m_c_d, "negb2m_c")
        ebqH = const_tile((128, CC, 8), BF16, ebqH_d, "ebqH")
        maskH64 = const_tile((8, CC, 128), F32R, maskH64_d, "maskH64")
        bvqbd = const_tile((128, CC, 128), F32, bvqbd_d, "bvqbd")
        ebqcol = const_tile((128, CC), F32, ebqcol_d, "ebqcol")

        # persistent across phases
        Eqc = cpool.tile([128, CC, L], BF16, tag="Eqc", name="Eqc")
        ctxbd = [cpool.tile([128, 128], BF16, tag=f"ctxbd{p}", name=f"ctxbd{p}")
                 for p in range(CC)]

        # ---------- Phase 1: exp(q) transpose + exp(k)/v fp8 + ctx ----------
        with ExitStack() as p1:
            lp1 = p1.enter_context(tc.tile_pool(name="lp1", bufs=2))
            pp1 = p1.enter_context(tc.tile_pool(name="pp1", bufs=3, space="PSUM"))
            ppt = p1.enter_context(tc.tile_pool(name="ppt", bufs=1, space="PSUM"))
            ppc = p1.enter_context(tc.tile_pool(name="ppc", bufs=1, space="PSUM"))

            ctxps = [ppc.tile([128, 129], F32, tag=f"ctx{p}", name=f"ctxps{p}")
                     for p in range(CC)]

            for lt in range(NL1):
                sl = slice(lt * 128, (lt + 1) * 128)
                half = lt % 2
                z1c = lp1.tile([128, CC, 128], F32R, tag="z1c")
                nc.sync.dma_start(z1c[:], z1r[:, :, sl])
                z2c = lp1.tile([128, CC, 128], F32R, tag="z2c")
                nc.sync.dma_start(z2c[:], z2r[:, :, sl])
                z18 = lp1.tile([128, CC, 128], FP8, tag="z18")
                pts(z18[:], z1c[:].bitcast(F32), 1.0, None, AluOpType.mult)
                z28 = lp1.tile([128, CC, 128], FP8, tag="z28")
                pts(z28[:], z2c[:].bitcast(F32), 1.0, None, AluOpType.mult)

                # qT [l,128][o,512] = z1^T Wq^T (x32 fp8 scale)
                qps = pp1.tile([128, 512], F32, tag="qkv", name="qps")
                mm(qps[:], z18[:, 0:2, :], Wq8T[:, 0:2, :], start=True, stop=False,
                   perf_mode=DR)
                mm(qps[:], z18[:, 2:4, :], Wq8T[:, 2:4, :], start=False, stop=True,
                   perf_mode=DR)
                Eq = lp1.tile([128, 512], BF16, tag="Eq")
                act(Eq[:], qps[:], AFT.Exp, scale=1.0 / WS)
                tps = ppt.tile([128, 512], BF16, tag="tp", name="tps")
                for cc in range(CC):
                    cs = slice(cc * 128, (cc + 1) * 128)
                    nc.tensor.transpose(tps[:, cs], Eq[:, cs], ident[:])
                nc.vector.tensor_copy(
                    Eqc[:, :, sl],
                    tps[:].rearrange("p (cc x) -> p cc x", x=128))

                # kT, vT
                kps = pp1.tile([128, 512], F32, tag="qkv", name="kps")
                mm(kps[:], z28[:, 0:2, :], Wk8T[:, 0:2, :], start=True, stop=False,
                   perf_mode=DR)
                mm(kps[:], z28[:, 2:4, :], Wk8T[:, 2:4, :], start=False, stop=True,
                   perf_mode=DR)
                vps = pp1.tile([128, 512], F32, tag="qkv", name="vps")
                mm(vps[:], z28[:, 0:2, :], Wv8T[:, 0:2, :], start=True, stop=False,
                   perf_mode=DR)
                mm(vps[:], z28[:, 2:4, :], Wv8T[:, 2:4, :], start=False, stop=True,
                   perf_mode=DR)

                if half == 0:
                    Ek8 = lp1.tile([128, 2, 512], FP8, tag="Ek8", name="Ek8")
                    v8 = lp1.tile([128, 2, CC, 132], FP8, tag="v8", name="v8")
                    nc.vector.memset(v8[:, :, :, 128:129], 1.0)
                act(Ek8[:, half, :], kps[:], AFT.Exp, scale=1.0 / WS)
                ts(v8[:, half, :, 0:128],
                   vps[:].rearrange("p (pr x) -> p pr x", x=128),
                   1.0 / WS, None, AluOpType.mult)

                if half == 1:
                    for pr in range(CC):
                        mm(ctxps[pr][:], Ek8[:, :, pr * 128:(pr + 1) * 128],
                           v8[:, :, pr, 0:129],
                           start=(lt == 1), stop=(lt == NL1 - 1),
                           perf_mode=DR, skip_group_check=True)

            # finalize ctx: rows / S, * e^bq, + e^bq*bv block-diag, -> bf16
            for pr in range(CC):
                rs = lp1.tile([128, 1], F32, tag="rs")
                nc.vector.reciprocal(rs[:], ctxps[pr][:, 128:129])
                rse = lp1.tile([128, 1], F32, tag="rse")
                tt(rse[:], rs[:], ebqcol[:, pr:pr + 1], AluOpType.mult)
                nc.vector.memset(ctxbd[pr][:], 0.0)
                stt(ctxbd[pr][0:64, 0:64], ctxps[pr][0:64, 0:64], rse[0:64, :],
                    bvqbd[0:64, pr, 0:64], AluOpType.mult, AluOpType.add)
                stt(ctxbd[pr][64:128, 64:128], ctxps[pr][64:128, 64:128],
                    rse[64:128, :], bvqbd[64:128, pr, 64:128],
                    AluOpType.mult, AluOpType.add)

        # ---------- Phase 2: apply + reprojection + LN1/FFN/LN2 ----------
        with ExitStack() as p2:
            lp2 = p2.enter_context(tc.tile_pool(name="lp2", bufs=2))
            pgen = p2.enter_context(tc.tile_pool(name="pgen", bufs=2, space="PSUM"))
            pfps = p2.enter_context(tc.tile_pool(name="pfps", bufs=2, space="PSUM"))
            pf2 = p2.enter_context(tc.tile_pool(name="pf2", bufs=2, space="PSUM"))
            pmu2 = p2.enter_context(tc.tile_pool(name="pmu2", bufs=1, space="PSUM"))
            pv2 = p2.enter_context(tc.tile_pool(name="pv2", bufs=1, space="PSUM"))

            for lt in range(NL2):
                sl = slice(lt * 512, (lt + 1) * 512)
                z1res = lp2.tile([128, CC, 512], F32R, tag="z1res")
                nc.sync.dma_start(z1res[:], z1r[:, :, sl])

                # Sq~[h,tau] = sum_k e^{bq[k]} Eq[k,tau]
                sqps = pgen.tile([128, 512], F32, tag="gen", name="sqps")
                for pr in range(CC):
                    mm(sqps[0:8, :], ebqH[:, pr, :], Eqc[:, pr, sl],
                       start=(pr == 0), stop=(pr == CC - 1))
                rq = lp2.tile([8, 512], F32R, tag="row", bufs=3)
                with nc.allow_low_precision(reason="f32r rounding is plenty for softmax norm"):
                    nc.vector.reciprocal(rq[:], sqps[0:8, :])

                # apply attention + normalize (x64) -> fp8 pair tiles
                att8A = lp2.tile([128, 2, 512], FP8, tag="att8A", name="att8A")
                att8B = lp2.tile([128, 2, 512], FP8, tag="att8B", name="att8B")
                for pr in range(CC):
                    rqbps = pgen.tile([128, 512], F32, tag="gen", name="rqbps")
                    mm(rqbps[:], maskH64[:, pr, :], rq[:],
                       start=True, stop=True)
                    rqbs = lp2.tile([128, 512], F32, tag="rqbs", bufs=2)
                    if pr % 2 == 0:
                        act(rqbs[:], rqbps[:], AFT.Copy)
                    else:
                        nc.vector.tensor_copy(rqbs[:], rqbps[:])
                    aps = pgen.tile([128, 512], F32, tag="gen", name="aps")
                    mm(aps[:], ctxbd[pr][:], Eqc[:, pr, sl], start=True, stop=True)
                    dst = att8A[:, pr, :] if pr < 2 else att8B[:, pr - 2, :]
                    tt(dst, aps[:], rqbs[:], AluOpType.mult)

                # z = (Wr8 @ att8) / (WS*AS) + z1   (br deferred into biases)
                zt = lp2.tile([128, CC, 512], F32R, tag="zt", name="zt")
                for oc in range(CC):
                    ocs = slice(oc * 128, (oc + 1) * 128)
                    zps = pgen.tile([128, 512], F32, tag="gen", name="zps")
                    mm(zps[:], Wr8T[:, 0:2, ocs], att8A[:], start=True, stop=False,
                       perf_mode=DR)
                    mm(zps[:], Wr8T[:, 2:4, ocs], att8B[:], start=False, stop=True,
                       perf_mode=DR)
                    stt(zt[:, oc, :], zps[:], SZ,
                        z1res[:, oc, :].bitcast(F32),
                        AluOpType.mult, AluOpType.add)

                # LN1 stats; true z = zt + br
                mups = pgen.tile([128, 512], F32, tag="gen", name="mups")
                for cc in range(CC):
                    mm(mups[0:1, :], inv512[:], zt[:, cc, :],
                       start=(cc == 0), stop=(cc == CC - 1))
                e2ps = pgen.tile([128, 512], F32, tag="gen", name="e2ps")
                for cc in range(CC):
                    zsq = lp2.tile([128, 512], F32R, tag="sqt", bufs=2)
                    act(zsq[:], zt[:, cc, :].bitcast(F32), AFT.Square, bias=br_c[:, cc:cc + 1])
                    mm(e2ps[0:1, :], inv512[:], zsq[:],
                       start=(cc == 0), stop=(cc == CC - 1))
                musq = lp2.tile([1, 512], F32, tag="row", bufs=3)
                act(musq[:], mups[0:1, :], AFT.Square, bias=brm_c[0:1, :])
                varrow = lp2.tile([1, 512], F32, tag="row", bufs=3)
                tt(varrow[:], e2ps[0:1, :], musq[:], AluOpType.subtract)
                lnv = lp2.tile([1, 512], F32, tag="row", bufs=3)
                act(lnv[:], varrow[:], AFT.Ln, bias=eps_c[0:1, :])
                rsig = lp2.tile([1, 512], F32R, tag="row", bufs=3)
                act(rsig[:], lnv[:], AFT.Exp, scale=-0.5)
                rhs2row = lp2.tile([1, 512], F32R, tag="rhs2row")
                stt(rhs2row[:], mups[0:1, :], brm_c[0:1, :], rsig[:].bitcast(F32),
                    AluOpType.add, AluOpType.mult)
                bcps = pgen.tile([128, 512], F32, tag="gen", name="bcps")
                mm(bcps[:], ones1x128[:], rsig[:], start=True,
                   stop=True)
                zs = lp2.tile([128, CC, 512], F32R, tag="zs", name="zs")
                for cc in range(CC):
                    stt(zs[:, cc, :], zt[:, cc, :].bitcast(F32),
                        br_c[:, cc:cc + 1], bcps[:],
                        AluOpType.add, AluOpType.mult)

                # FFN1 + ELU; mu2 row accumulates on its own bank
                mu2ps = pmu2.tile([128, 512], F32, tag="mu2", name="mu2ps")
                he = []
                for j in range(8):
                    js = slice(j * 128, (j + 1) * 128)
                    fps = pfps.tile([128, 512], F32, tag="fps", name="fps")
                    for cc in range(CC):
                        mm(fps[:], W1gTt[:, cc, js], zs[:, cc, :],
                           start=(cc == 0), stop=False)
                    mm(fps[:], u1neg[:, js], rhs2row[:], start=False, stop=True)
                    E = lp2.tile([128, 512], F32, tag="E", bufs=2, name="E")
                    act(E[:], fps[:], AFT.Exp, bias=w1bbc[:, j:j + 1])
                    rh = lp2.tile([128, 512], BF16, tag="rh", bufs=2, name="rh")
                    if j % 2 == 0:
                        act(rh[:], fps[:], AFT.Relu, bias=w1bbc[:, j:j + 1])
                    else:
                        ts(rh[:], fps[:], w1bbc[:, j:j + 1], 0.0,
                           AluOpType.add, AluOpType.max)
                    Em = lp2.tile([128, 512], F32, tag="Em", bufs=2, name="Em")
                    pts(Em[:], E[:], 1.0, -1.0, AluOpType.min, AluOpType.add)
                    hej = lp2.tile([128, 512], BF16, tag=f"he{j}", name=f"he{j}")
                    ptt(hej[:], Em[:], rh[:], AluOpType.add)
                    he.append(hej)
                    mm(mu2ps[0:1, :], u2ct[:, j:j + 1], hej[:],
                       start=(j == 0), stop=(j == 7), skip_group_check=True)

                negmu2 = lp2.tile([1, 512], F32R, tag="negmu2", name="negmu2")
                ts(negmu2[:], mu2ps[0:1, :], -1.0, negb2m_c[0:1, :],
                   AluOpType.mult, AluOpType.add)

                # FFN2 (o-outer) + LN2 var row
                v2ps = pv2.tile([128, 512], F32, tag="v2", name="v2ps")
                yg = []
                for oc in range(CC):
                    ocs = slice(oc * 128, (oc + 1) * 128)
                    f2 = pf2.tile([128, 512], F32, tag="f2", name="f2")
                    for j in range(8):
                        mm(f2[:], W2gTt[:, j, ocs], he[j][:],
                           start=(j == 0), stop=False)
                    mm(f2[:], g2b2row[:, ocs], ones_row[:], start=False, stop=False)
                    mm(f2[:], g2row[:, ocs], negmu2[:], start=False, stop=True)
                    y = lp2.tile([128, 512], F32, tag=f"yg{oc}", bufs=1, name=f"yg{oc}")
                    act(y[:], f2[:], AFT.Copy)
                    sq2 = lp2.tile([128, 512], F32R, tag="sqt", bufs=2)
                    act(sq2[:], f2[:], AFT.Square)
                    mm(v2ps[0:1, :], ivgt[:, oc:oc + 1], sq2[:],
                       start=(oc == 0), stop=(oc == CC - 1), skip_group_check=True)
                    yg.append(y)
                lnv2 = lp2.tile([1, 512], F32, tag="row", bufs=3)
                act(lnv2[:], v2ps[0:1, :], AFT.Ln, bias=eps_c[0:1, :])
                rs2 = lp2.tile([1, 512], F32R, tag="row", bufs=3)
                act(rs2[:], lnv2[:], AFT.Exp, scale=-0.5)
                bc2ps = pgen.tile([128, 512], F32, tag="gen", name="bc2ps")
                mm(bc2ps[:], ones1x128[:], rs2[:], start=True,
                   stop=True)
                for oc in range(CC):
                    ot = lp2.tile([128, 512], F32, tag="ot", bufs=3, name=f"ot{oc}")
                    tt(ot[:], yg[oc][:], bc2ps[:], AluOpType.mult)
                    pts(ot[:], ot[:], be2_c[:, oc:oc + 1], None,
                        AluOpType.add)
                    nc.sync.dma_start(outd[oc * 128:(oc + 1) * 128, sl], ot[:])

    nc.compile()
    return nc


def _prep_consts(Wq, bq, Wk, bk, Wv, bv, Wr, br, g1, be1, W1, b1, W2, b2, g2, be2):
    import ml_dtypes
    f = np.float32
    fp8 = ml_dtypes.float8_e4m3

    def chunkT(a, n):          # [n*128, m] -> [128, n, m]
        return np.ascontiguousarray(a.reshape(n, 128, -1).transpose(1, 0, 2))

    def colsT(v, n):           # [n*128] -> [128, n]
        return np.ascontiguousarray(v.reshape(n, 128).T)

    WqT = np.ascontiguousarray(Wq.T, dtype=f)
    WkT = np.ascontiguousarray(Wk.T, dtype=f)
    WvT = np.ascontiguousarray(Wv.T, dtype=f)
    WrT = np.ascontiguousarray(Wr.T, dtype=f)
    W1g = (W1 * g1[None, :]).astype(f)                              # [1024, c]
    W1gT = np.ascontiguousarray(W1g.T)
    W2g = (W2 * g2[:, None]).astype(f)
    W2gT = np.ascontiguousarray(W2g.T)                              # [1024h, c]
    u1neg = -W1g.sum(axis=1).astype(f)
    w1bb = (W1 @ be1 + b1).astype(f)
    u2 = (W2.sum(axis=0) / 512.0).astype(f)
    ivg = (1.0 / (512.0 * g2 * g2)).astype(f)
    b2mean = float(np.mean(b2))
    brmean = float(np.mean(br))

    ebq = np.exp(bq.astype(np.float64)).astype(f)                   # [512]
    # ebqH[p, pr, h] = e^{bq[pr*128+p]} if h == 2*pr + (p>=64)
    ebqH = np.zeros((128, CC, 8), dtype=f)
    for pr in range(CC):
        for p in range(128):
            ebqH[p, pr, 2 * pr + (p >= 64)] = ebq[pr * 128 + p]
    # maskH64[h, pr, v] = 64 if head-of(pr,v) == h
    maskH64 = np.zeros((8, CC, 128), dtype=f)
    for pr in range(CC):
        for v in range(128):
            maskH64[2 * pr + (v >= 64), pr, v] = AS
    # bvqbd[p, pr, v] = e^{bq[pr*128+p]} * bv[pr*128+v] * same-head(p, v)
    bvqbd = np.zeros((128, CC, 128), dtype=f)
    for pr in range(CC):
        for p in range(128):
            lo = 0 if p < 64 else 64
            bvqbd[p, pr, lo:lo + 64] = ebq[pr * 128 + p] * bv[pr * 128 + lo:
                                                              pr * 128 + lo + 64]

    return {
        "Wq8T": chunkT(WqT * WS, CC).astype(fp8),
        "Wk8T": chunkT(WkT * WS, CC).astype(fp8),
        "Wv8T": chunkT(WvT * WS, CC).astype(fp8),
        "Wr8T": chunkT(WrT * WS, CC).astype(fp8),
        "W1gTt": chunkT(W1gT, CC),
        "W2gTt": chunkT(W2gT, 8).astype(ml_dtypes.bfloat16),
        "u1neg": u1neg.reshape(1, 1024),
        "w1bbc": colsT(w1bb, 8),
        "u2ct": colsT(u2, 8).astype(ml_dtypes.bfloat16),
        "g2b2row": (g2 * b2).astype(f).reshape(1, 512),
        "g2row": g2.astype(f).reshape(1, 512),
        "ones_row": np.ones((1, 512), dtype=f),
        "ivgt": colsT(ivg, CC),
        "inv512": np.full((128, 1), 1.0 / 512.0, dtype=f),
        "ones1x128": np.ones((1, 128), dtype=f),
        "ident": np.eye(128, dtype=ml_dtypes.bfloat16),
        "br_c": colsT(br.astype(f), CC),
        "be2_c": colsT(be2.astype(f), CC),
        "eps_c": np.full((128, 1), EPS, dtype=f),
        "brm_c": np.full((128, 1), brmean, dtype=f),
        "negb2m_c": np.full((128, 1), -b2mean, dtype=f),
        "ebqH": ebqH.astype(ml_dtypes.bfloat16),
        "maskH64": maskH64,
        "bvqbd": bvqbd,
        "ebqcol": colsT(ebq, CC),
    }, brmean, b2mean


def kernel(**inputs):
    global LAST_RESULT
    z1 = np.asarray(inputs["z1"], dtype=np.float32)
    z2 = np.asarray(inputs["z2"], dtype=np.float32)
    consts, brmean, b2mean = _prep_consts(
        *[np.asarray(inputs[k], dtype=np.float32) for k in
          ["Wq", "bq", "Wk", "bk", "Wv", "bv", "Wr", "br", "g1", "be1",
           "W1", "b1", "W2", "b2", "g2", "be2"]])

    key = "prog"
    if key not in _CACHE:
        _CACHE[key] = _build_program()
    nc = _CACHE[key]

    in_maps = []
    for b in range(B):
        m = dict(consts)
        m["z1"] = np.ascontiguousarray(z1[b])
        m["z2"] = np.ascontiguousarray(z2[b])
        in_maps.append(m)

    import os
    trace = bool(int(os.environ.get("KERNEL_TRACE", "0")))
    res = run_bass_kernel_spmd(nc, in_maps, list(range(B)), trace=trace)
    LAST_RESULT = res
    out = np.stack([res.results[b]["out"] for b in range(B)], axis=0)
    return out.astype(np.float32)


# revision 12
# speedup vs baseline: 1.4435x; 1.0773x over previous
"""CACombiner Trainium2 kernel: conv-projected efficient attention + FFN.

Data-parallel over batch: 8 batch elements -> 8 NeuronCores, identical SPMD
program per core.

v2: the attention path (q/k/v projections, ctx accumulation, reprojection)
runs in fp8e4m3 with DoubleRow matmuls (K=256 per instruction at 0.5
cyc/row).  The attention branch contributes ~0.3% of the residual stream, so
fp8 noise there is negligible.  Biases are folded exactly:
  - bk cancels in the key softmax (constant along L per channel);
  - bv folds into the normalized ctx (+bv[v] per column);
  - bq folds as e^{bq} row weights into ctx2 and the deferred softmax-q
    normalization sum.
Softmax-q normalization is deferred to phase 2 (unnormalized exp(q) kept
channels-first in bf16; per-(head,token) 1/sum applied after the ctx apply).
LayerNorm rsqrt = exp(-0.5*ln(var+eps)) keeps every activation in one act
table set (no table reloads).  ELU = min(e^x - 1, max(x, 0)).
FFN matmuls keep full-precision f32r weights with bf16 moving operands.
"""
import sys
sys.path.insert(0, "/opt/trn_rl_repo")
from contextlib import ExitStack

import numpy as np

import concourse.bass as bass
import concourse.tile as tile
from concourse import mybir, bacc
from concourse.bass_utils import run_bass_kernel_spmd
from concourse.alu_op_type import AluOpType

F32 = mybir.dt.float32
F32R = mybir.dt.float32r
BF16 = mybir.dt.bfloat16
FP8 = mybir.dt.float8e4
AFT = mybir.ActivationFunctionType
DR = mybir.MatmulPerfMode.DoubleRow

B, C, L = 8, 512, 4096
H, DK = 8, 64
EPS = 1e-5
CC = C // 128           # 4 channel chunks
NL1 = L // 128          # 32 phase-1 l-tiles
NL2 = L // 512          # 8 phase-2 l-tiles
WS = 32.0               # fp8 weight scale for Wq/Wk/Wv/Wr
AS = 64.0               # att fp8 scale
SZ = 1.0 / (WS * AS)    # undo both scales after the Wr matmul

_CACHE = {}
LAST_RESULT = None


def _build_program():
    nc = bacc.Bacc("TRN2", target_bir_lowering=False, debug=False)

    def din(name, shape, dtype):
        return nc.dram_tensor(name, list(shape), dtype, kind="ExternalInput").ap()

    z1d = din("z1", (C, L), F32R)
    z2d = din("z2", (C, L), F32R)
    Wq8T_d = din("Wq8T", (128, CC, 512), FP8)
    Wk8T_d = din("Wk8T", (128, CC, 512), FP8)
    Wv8T_d = din("Wv8T", (128, CC, 512), FP8)
    Wr8T_d = din("Wr8T", (128, CC, 512), FP8)
    W1gTt_d = din("W1gTt", (128, CC, 1024), F32R)
    W2gTt_d = din("W2gTt", (128, 8, 512), BF16)
    u1neg_d = din("u1neg", (1, 1024), F32R)
    w1bbc_d = din("w1bbc", (128, 8), F32)
    u2ct_d = din("u2ct", (128, 8), BF16)
    g2b2row_d = din("g2b2row", (1, 512), F32R)
    g2row_d = din("g2row", (1, 512), F32R)
    ones_row_d = din("ones_row", (1, 512), F32R)
    ivgt_d = din("ivgt", (128, CC), F32R)
    inv512_d = din("inv512", (128, 1), F32R)
    ones1x128_d = din("ones1x128", (1, 128), F32R)
    ident_d = din("ident", (128, 128), BF16)
    br_c_d = din("br_c", (128, CC), F32)
    be2_c_d = din("be2_c", (128, CC), F32)
    eps_c_d = din("eps_c", (128, 1), F32)
    brm_c_d = din("brm_c", (128, 1), F32)
    negb2m_c_d = din("negb2m_c", (128, 1), F32)
    ebqH_d = din("ebqH", (128, CC, 8), BF16)
    maskH64_d = din("maskH64", (8, CC, 128), F32R)
    bvqbd_d = din("bvqbd", (128, CC, 128), F32)
    ebqcol_d = din("ebqcol", (128, CC), F32)
    outd = nc.dram_tensor("out", [C, L], F32, kind="ExternalOutput").ap()

    z1r = z1d.rearrange("(cc p) l -> p cc l", p=128)
    z2r = z2d.rearrange("(cc p) l -> p cc l", p=128)

    mm = nc.tensor.matmul
    tt = nc.vector.tensor_tensor
    ts = nc.vector.tensor_scalar
    stt = nc.vector.scalar_tensor_tensor
    ptt = nc.gpsimd.tensor_tensor
    pts = nc.gpsimd.tensor_scalar
    pstt = nc.gpsimd.scalar_tensor_tensor
    act = nc.scalar.activation

    with tile.TileContext(nc) as tc, ExitStack() as ctx:
        cpool = ctx.enter_context(tc.tile_pool(name="consts", bufs=1))

        deferred_dmas = []

        def const_tile(shape, dtype, src, tag, defer=True):
            t = cpool.tile(list(shape), dtype, tag=tag, name=tag)
            if defer:
                deferred_dmas.append((t, src))
            else:
                nc.sync.dma_start(t[:], src)
            return t

        # pre-load the one act table that covers Exp/Ln/Square/Copy/Relu so
        # the insertion pass never needs another table set
        from concourse.hw_specs import get_activation_tables
        _tabs = list(get_activation_tables(nc.m.arch).keys())
        nc.scalar.add_instruction(mybir.InstLoadActFuncSet(
            name=f"I-{nc.next_id()}", ins=[], outs=[],
            act_func_set_id=_tabs.index("natural_log_exp_and_others")))

        Wq8T = const_tile((128, CC, 512), FP8, Wq8T_d, "Wq8T", defer=False)
        Wk8T = const_tile((128, CC, 512), FP8, Wk8T_d, "Wk8T", defer=False)
        Wv8T = const_tile((128, CC, 512), FP8, Wv8T_d, "Wv8T", defer=False)
        Wr8T = const_tile((128, CC, 512), FP8, Wr8T_d, "Wr8T")
        W1gTt = const_tile((128, CC, 1024), F32R, W1gTt_d, "W1gTt")
        W2gTt = const_tile((128, 8, 512), BF16, W2gTt_d, "W2gTt")
        u1neg = const_tile((1, 1024), F32R, u1neg_d, "u1neg")
        w1bbc = const_tile((128, 8), F32, w1bbc_d, "w1bbc")
        u2ct = const_tile((128, 8), BF16, u2ct_d, "u2ct")
        g2b2row = const_tile((1, 512), F32R, g2b2row_d, "g2b2row")
        g2row = const_tile((1, 512), F32R, g2row_d, "g2row")
        ones_row = const_tile((1, 512), F32R, ones_row_d, "ones_row")
        ivgt = const_tile((128, CC), F32R, ivgt_d, "ivgt")
        inv512 = const_tile((128, 1), F32R, inv512_d, "inv512")
        ones1x128 = const_tile((1, 128), F32R, ones1x128_d, "ones1x128")
        ident = const_tile((128, 128), BF16, ident_d, "ident", defer=False)
        br_c = const_tile((128, CC), F32, br_c_d, "br_c")
        be2_c = const_tile((128, CC), F32, be2_c_d, "be2_c")
        eps_c = const_tile((128, 1), F32, eps_c_d, "eps_c")
        brm_c = const_tile((128, 1), F32, brm_c_d, "brm_c")
        negb2m_c = const_tile((128, 1), F32, negb2m_c_d, "negb2m_c")
        ebqH = const_tile((128, CC, 8), BF16, ebqH_d, "ebqH")
        maskH64 = const_tile((8, CC, 128), F32R, maskH64_d, "maskH64")
        bvqbd = const_tile((128, CC, 128), F32, bvqbd_d, "bvqbd")
        ebqcol = const_tile((128, CC), F32, ebqcol_d, "ebqcol")

        # persistent across phases
        Eqc = cpool.tile([128, CC, L], BF16, tag="Eqc", name="Eqc")
        ctxbd = [cpool.tile([128, 128], BF16, tag=f"ctxbd{p}", name=f"ctxbd{p}")
                 for p in range(CC)]

        # ---------- Phase 1: exp(q) transpose + exp(k)/v fp8 + ctx ----------
        with ExitStack() as p1:
            lp1 = p1.enter_context(tc.tile_pool(name="lp1", bufs=2))
            pp1 = p1.enter_context(tc.tile_pool(name="pp1", bufs=3, space="PSUM"))
            ppt = p1.enter_context(tc.tile_pool(name="ppt", bufs=1, space="PSUM"))
            ppc = p1.enter_context(tc.tile_pool(name="ppc", bufs=1, space="PSUM"))

            ctxps = [ppc.tile([128, 129], F32, tag=f"ctx{p}", name=f"ctxps{p}")
                     for p in range(CC)]

            for lt in range(NL1):
                sl = slice(lt * 128, (lt + 1) * 128)
                half = lt % 2
                z18 = lp1.tile([128, CC, 128], FP8, tag="z18")
                nc.gpsimd.dma_start(z18[:], z1r[:, :, sl])
                z28 = lp1.tile([128, CC, 128], FP8, tag="z28")
                nc.gpsimd.dma_start(z28[:], z2r[:, :, sl])

                # qT [l,128][o,512] = z1^T Wq^T (x32 fp8 scale)
                qps = pp1.tile([128, 512], F32, tag="qkv", name="qps")
                mm(qps[:], z18[:, 0:2, :], Wq8T[:, 0:2, :], start=True, stop=False,
                   perf_mode=DR)
                mm(qps[:], z18[:, 2:4, :], Wq8T[:, 2:4, :], start=False, stop=True,
                   perf_mode=DR)
                Eq = lp1.tile([128, 512], BF16, tag="Eq")
                act(Eq[:], qps[:], AFT.Exp, scale=1.0 / WS)
                tps = ppt.tile([128, 512], BF16, tag="tp", name="tps")
                for cc in range(CC):
                    cs = slice(cc * 128, (cc + 1) * 128)
                    nc.tensor.transpose(tps[:, cs], Eq[:, cs], ident[:])
                nc.vector.tensor_copy(
                    Eqc[:, :, sl],
                    tps[:].rearrange("p (cc x) -> p cc x", x=128))

                # kT, vT
                kps = pp1.tile([128, 512], F32, tag="qkv", name="kps")
                mm(kps[:], z28[:, 0:2, :], Wk8T[:, 0:2, :], start=True, stop=False,
                   perf_mode=DR)
                mm(kps[:], z28[:, 2:4, :], Wk8T[:, 2:4, :], start=False, stop=True,
                   perf_mode=DR)
                vps = pp1.tile([128, 512], F32, tag="qkv", name="vps")
                mm(vps[:], z28[:, 0:2, :], Wv8T[:, 0:2, :], start=True, stop=False,
                   perf_mode=DR)
                mm(vps[:], z28[:, 2:4, :], Wv8T[:, 2:4, :], start=False, stop=True,
                   perf_mode=DR)

                if half == 0:
                    Ek8 = lp1.tile([128, 2, 512], FP8, tag="Ek8", name="Ek8")
                    v8 = lp1.tile([128, 2, CC, 132], FP8, tag="v8", name="v8")
                    nc.vector.memset(v8[:, :, :, 128:129], 1.0)
                act(Ek8[:, half, :], kps[:], AFT.Exp, scale=1.0 / WS)
                ts(v8[:, half, :, 0:128],
                   vps[:].rearrange("p (pr x) -> p pr x", x=128),
                   1.0 / WS, None, AluOpType.mult)

                if half == 1:
                    for pr in range(CC):
                        mm(ctxps[pr][:], Ek8[:, :, pr * 128:(pr + 1) * 128],
                           v8[:, :, pr, 0:129],
                           start=(lt == 1), stop=(lt == NL1 - 1),
                           perf_mode=DR, skip_group_check=True)

            for _t, _src in deferred_dmas:
                nc.sync.dma_start(_t[:], _src)

            # finalize ctx: rows / S, * e^bq, + e^bq*bv block-diag, -> bf16
            for pr in range(CC):
                rs = lp1.tile([128, 1], F32, tag="rs")
                nc.vector.reciprocal(rs[:], ctxps[pr][:, 128:129])
                rse = lp1.tile([128, 1], F32, tag="rse")
                tt(rse[:], rs[:], ebqcol[:, pr:pr + 1], AluOpType.mult)
                nc.vector.memset(ctxbd[pr][:], 0.0)
                stt(ctxbd[pr][0:64, 0:64], ctxps[pr][0:64, 0:64], rse[0:64, :],
                    bvqbd[0:64, pr, 0:64], AluOpType.mult, AluOpType.add)
                stt(ctxbd[pr][64:128, 64:128], ctxps[pr][64:128, 64:128],
                    rse[64:128, :], bvqbd[64:128, pr, 64:128],
                    AluOpType.mult, AluOpType.add)

        # ---------- Phase 2: apply + reprojection + LN1/FFN/LN2 ----------
        with ExitStack() as p2:
            lp2 = p2.enter_context(tc.tile_pool(name="lp2", bufs=2))
            pgen = p2.enter_context(tc.tile_pool(name="pgen", bufs=2, space="PSUM"))
            pfps = p2.enter_context(tc.tile_pool(name="pfps", bufs=2, space="PSUM"))
            pf2 = p2.enter_context(tc.tile_pool(name="pf2", bufs=2, space="PSUM"))
            pmuv = p2.enter_context(tc.tile_pool(name="pmuv", bufs=2, space="PSUM"))

            for lt in range(NL2):
                sl = slice(lt * 512, (lt + 1) * 512)
                z1res = lp2.tile([128, CC, 512], F32R, tag="z1res")
                nc.sync.dma_start(z1res[:], z1r[:, :, sl])

                # Sq~[h,tau] = sum_k e^{bq[k]} Eq[k,tau]
                sqps = pgen.tile([128, 512], F32, tag="gen", name="sqps")
                for pr in range(CC):
                    mm(sqps[0:8, :], ebqH[:, pr, :], Eqc[:, pr, sl],
                       start=(pr == 0), stop=(pr == CC - 1))
                rq = lp2.tile([8, 512], F32R, tag="row", bufs=3)
                with nc.allow_low_precision(reason="f32r rounding is plenty for softmax norm"):
                    nc.vector.reciprocal(rq[:], sqps[0:8, :])

                # apply attention + normalize (x64) -> fp8 pair tiles
                att8A = lp2.tile([128, 2, 512], FP8, tag="att8A", name="att8A")
                att8B = lp2.tile([128, 2, 512], FP8, tag="att8B", name="att8B")
                for pr in range(CC):
                    rqbps = pgen.tile([128, 512], F32, tag="gen", name="rqbps")
                    mm(rqbps[:], maskH64[:, pr, :], rq[:],
                       start=True, stop=True)
                    rqbs = lp2.tile([128, 512], F32, tag="rqbs", bufs=2)
                    if pr % 2 == 0:
                        act(rqbs[:], rqbps[:], AFT.Copy)
                    else:
                        nc.vector.tensor_copy(rqbs[:], rqbps[:])
                    aps = pgen.tile([128, 512], F32, tag="gen", name="aps")
                    mm(aps[:], ctxbd[pr][:], Eqc[:, pr, sl], start=True, stop=True)
                    dst = att8A[:, pr, :] if pr < 2 else att8B[:, pr - 2, :]
                    tt(dst, aps[:], rqbs[:], AluOpType.mult)

                # z = (Wr8 @ att8) / (WS*AS) + z1   (br deferred into biases)
                zt = lp2.tile([128, CC, 512], F32R, tag="zt", name="zt")
                for oc in range(CC):
                    ocs = slice(oc * 128, (oc + 1) * 128)
                    zps = pgen.tile([128, 512], F32, tag="gen", name="zps")
                    mm(zps[:], Wr8T[:, 0:2, ocs], att8A[:], start=True, stop=False,
                       perf_mode=DR)
                    mm(zps[:], Wr8T[:, 2:4, ocs], att8B[:], start=False, stop=True,
                       perf_mode=DR)
                    stt(zt[:, oc, :], zps[:], SZ,
                        z1res[:, oc, :].bitcast(F32),
                        AluOpType.mult, AluOpType.add)

                # LN1 stats; true z = zt + br
                mups = pgen.tile([128, 512], F32, tag="gen", name="mups")
                for cc in range(CC):
                    mm(mups[0:1, :], inv512[:], zt[:, cc, :],
                       start=(cc == 0), stop=(cc == CC - 1))
                e2ps = pgen.tile([128, 512], F32, tag="gen", name="e2ps")
                for cc in range(CC):
                    zsq = lp2.tile([128, 512], F32R, tag="sqt", bufs=2)
                    act(zsq[:], zt[:, cc, :].bitcast(F32), AFT.Square, bias=br_c[:, cc:cc + 1])
                    mm(e2ps[0:1, :], inv512[:], zsq[:],
                       start=(cc == 0), stop=(cc == CC - 1))
                musq = lp2.tile([1, 512], F32, tag="row", bufs=3)
                act(musq[:], mups[0:1, :], AFT.Square, bias=brm_c[0:1, :])
                varrow = lp2.tile([1, 512], F32, tag="row", bufs=3)
                tt(varrow[:], e2ps[0:1, :], musq[:], AluOpType.subtract)
                lnv = lp2.tile([1, 512], F32, tag="row", bufs=3)
                act(lnv[:], varrow[:], AFT.Ln, bias=eps_c[0:1, :])
                rsig = lp2.tile([1, 512], F32R, tag="row", bufs=3)
                act(rsig[:], lnv[:], AFT.Exp, scale=-0.5)
                rhs2row = lp2.tile([1, 512], F32R, tag="rhs2row")
                stt(rhs2row[:], mups[0:1, :], brm_c[0:1, :], rsig[:].bitcast(F32),
                    AluOpType.add, AluOpType.mult)
                bcps = pgen.tile([128, 512], F32, tag="gen", name="bcps")
                mm(bcps[:], ones1x128[:], rsig[:], start=True,
                   stop=True)
                invsb = lp2.tile([128, 512], F32, tag="invsb", name="invsb")
                act(invsb[:], bcps[:], AFT.Copy)
                zs = lp2.tile([128, CC, 512], F32R, tag="zs", name="zs")
                for cc in range(CC):
                    stt(zs[:, cc, :], zt[:, cc, :].bitcast(F32),
                        br_c[:, cc:cc + 1], invsb[:],
                        AluOpType.add, AluOpType.mult)

                # FFN1 + ELU; mu2 row accumulates on its own bank
                mu2ps = pmuv.tile([128, 512], F32, tag="muv", name="mu2ps")
                he = []
                for j in range(8):
                    js = slice(j * 128, (j + 1) * 128)
                    fps = pfps.tile([128, 512], F32, tag="fps", name="fps")
                    for cc in range(CC):
                        mm(fps[:], W1gTt[:, cc, js], zs[:, cc, :],
                           start=(cc == 0), stop=False)
                    mm(fps[:], u1neg[:, js], rhs2row[:], start=False, stop=True)
                    E = lp2.tile([128, 512], F32, tag="E", bufs=2, name="E")
                    act(E[:], fps[:], AFT.Exp, bias=w1bbc[:, j:j + 1])
                    rh = lp2.tile([128, 512], BF16, tag="rh", bufs=2, name="rh")
                    if j % 2 == 0:
                        act(rh[:], fps[:], AFT.Relu, bias=w1bbc[:, j:j + 1])
                    else:
                        ts(rh[:], fps[:], w1bbc[:, j:j + 1], 0.0,
                           AluOpType.add, AluOpType.max)
                    Em = lp2.tile([128, 512], F32, tag="Em", bufs=2, name="Em")
                    pts(Em[:], E[:], 1.0, -1.0, AluOpType.min, AluOpType.add)
                    hej = lp2.tile([128, 512], BF16, tag=f"he{j}", name=f"he{j}")
                    ptt(hej[:], Em[:], rh[:], AluOpType.add)
                    he.append(hej)
                    mm(mu2ps[0:1, :], u2ct[:, j:j + 1], hej[:],
                       start=(j == 0), stop=(j == 7), skip_group_check=True)

                negmu2 = lp2.tile([1, 512], F32R, tag="negmu2", name="negmu2")
                ts(negmu2[:], mu2ps[0:1, :], -1.0, negb2m_c[0:1, :],
                   AluOpType.mult, AluOpType.add)

                # FFN2 (o-outer) + LN2 var row
                v2ps = pmuv.tile([128, 512], F32, tag="muv", name="v2ps")
                yg = []
                for oc in range(CC):
                    ocs = slice(oc * 128, (oc + 1) * 128)
                    f2 = pf2.tile([128, 512], F32, tag="f2", name="f2")
                    for j in range(8):
                        mm(f2[:], W2gTt[:, j, ocs], he[j][:],
                           start=(j == 0), stop=False)
                    mm(f2[:], g2b2row[:, ocs], ones_row[:], start=False, stop=False)
                    mm(f2[:], g2row[:, ocs], negmu2[:], start=False, stop=True)
                    y = lp2.tile([128, 512], F32, tag=f"yg{oc}", bufs=1, name=f"yg{oc}")
                    act(y[:], f2[:], AFT.Copy)
                    sq2 = lp2.tile([128, 512], F32R, tag="sqt", bufs=2)
                    act(sq2[:], f2[:], AFT.Square)
                    mm(v2ps[0:1, :], ivgt[:, oc:oc + 1], sq2[:],
                       start=(oc == 0), stop=(oc == CC - 1), skip_group_check=True)
                    yg.append(y)
                lnv2 = lp2.tile([1, 512], F32, tag="row", bufs=3)
                act(lnv2[:], v2ps[0:1, :], AFT.Ln, bias=eps_c[0:1, :])
                rs2 = lp2.tile([1, 512], F32R, tag="row", bufs=3)
                act(rs2[:], lnv2[:], AFT.Exp, scale=-0.5)
                bc2ps = pgen.tile([128, 512], F32, tag="gen", name="bc2ps")
                mm(bc2ps[:], ones1x128[:], rs2[:], start=True,
                   stop=True)
                invsb2 = lp2.tile([128, 512], F32, tag="invsb2", name="invsb2")
                act(invsb2[:], bc2ps[:], AFT.Copy)
                for oc in range(CC):
                    ot = lp2.tile([128, 512], F32, tag="ot", bufs=3, name=f"ot{oc}")
                    (tt if oc % 2 == 0 else ptt)(ot[:], yg[oc][:], invsb2[:],
                                                 AluOpType.mult)
                    (pts if oc % 2 == 0 else ts)(ot[:], ot[:],
                                                 be2_c[:, oc:oc + 1], None,
                                                 AluOpType.add)
                    nc.sync.dma_start(outd[oc * 128:(oc + 1) * 128, sl], ot[:])

    nc.compile()
    return nc


def _prep_consts(Wq, bq, Wk, bk, Wv, bv, Wr, br, g1, be1, W1, b1, W2, b2, g2, be2):
    import ml_dtypes
    f = np.float32
    fp8 = ml_dtypes.float8_e4m3

    def chunkT(a, n):          # [n*128, m] -> [128, n, m]
        return np.ascontiguousarray(a.reshape(n, 128, -1).transpose(1, 0, 2))

    def colsT(v, n):           # [n*128] -> [128, n]
        return np.ascontiguousarray(v.reshape(n, 128).T)

    WqT = np.ascontiguousarray(Wq.T, dtype=f)
    WkT = np.ascontiguousarray(Wk.T, dtype=f)
    WvT = np.ascontiguousarray(Wv.T, dtype=f)
    WrT = np.ascontiguousarray(Wr.T, dtype=f)
    W1g = (W1 * g1[None, :]).astype(f)                              # [1024, c]
    W1gT = np.ascontiguousarray(W1g.T)
    W2g = (W2 * g2[:, None]).astype(f)
    W2gT = np.ascontiguousarray(W2g.T)                              # [1024h, c]
    u1neg = -W1g.sum(axis=1).astype(f)
    w1bb = (W1 @ be1 + b1).astype(f)
    u2 = (W2.sum(axis=0) / 512.0).astype(f)
    ivg = (1.0 / (512.0 * g2 * g2)).astype(f)
    b2mean = float(np.mean(b2))
    brmean = float(np.mean(br))

    ebq = np.exp(bq.astype(np.float64)).astype(f)                   # [512]
    # ebqH[p, pr, h] = e^{bq[pr*128+p]} if h == 2*pr + (p>=64)
    ebqH = np.zeros((128, CC, 8), dtype=f)
    for pr in range(CC):
        for p in range(128):
            ebqH[p, pr, 2 * pr + (p >= 64)] = ebq[pr * 128 + p]
    # maskH64[h, pr, v] = 64 if head-of(pr,v) == h
    maskH64 = np.zeros((8, CC, 128), dtype=f)
    for pr in range(CC):
        for v in range(128):
            maskH64[2 * pr + (v >= 64), pr, v] = AS
    # bvqbd[p, pr, v] = e^{bq[pr*128+p]} * bv[pr*128+v] * same-head(p, v)
    bvqbd = np.zeros((128, CC, 128), dtype=f)
    for pr in range(CC):
        for p in range(128):
            lo = 0 if p < 64 else 64
            bvqbd[p, pr, lo:lo + 64] = ebq[pr * 128 + p] * bv[pr * 128 + lo:
                                                              pr * 128 + lo + 64]

    return {
        "Wq8T": chunkT(WqT * WS, CC).astype(fp8),
        "Wk8T": chunkT(WkT * WS, CC).astype(fp8),
        "Wv8T": chunkT(WvT * WS, CC).astype(fp8),
        "Wr8T": chunkT(WrT * WS, CC).astype(fp8),
        "W1gTt": chunkT(W1gT, CC),
        "W2gTt": chunkT(W2gT, 8).astype(ml_dtypes.bfloat16),
        "u1neg": u1neg.reshape(1, 1024),
        "w1bbc": colsT(w1bb, 8),
        "u2ct": colsT(u2, 8).astype(ml_dtypes.bfloat16),
        "g2b2row": (g2 * b2).astype(f).reshape(1, 512),
        "g2row": g2.astype(f).reshape(1, 512),
        "ones_row": np.ones((1, 512), dtype=f),
        "ivgt": colsT(ivg, CC),
        "inv512": np.full((128, 1), 1.0 / 512.0, dtype=f),
        "ones1x128": np.ones((1, 128), dtype=f),
        "ident": np.eye(128, dtype=ml_dtypes.bfloat16),
        "br_c": colsT(br.astype(f), CC),
        "be2_c": colsT(be2.astype(f), CC),
        "eps_c": np.full((128, 1), EPS, dtype=f),
        "brm_c": np.full((128, 1), brmean, dtype=f),
        "negb2m_c": np.full((128, 1), -b2mean, dtype=f),
        "ebqH": ebqH.astype(ml_dtypes.bfloat16),
        "maskH64": maskH64,
        "bvqbd": bvqbd,
        "ebqcol": colsT(ebq, CC),
    }, brmean, b2mean


def kernel(**inputs):
    global LAST_RESULT
    z1 = np.asarray(inputs["z1"], dtype=np.float32)
    z2 = np.asarray(inputs["z2"], dtype=np.float32)
    consts, brmean, b2mean = _prep_consts(
        *[np.asarray(inputs[k], dtype=np.float32) for k in
          ["Wq", "bq", "Wk", "bk", "Wv", "bv", "Wr", "br", "g1", "be1",
           "W1", "b1", "W2", "b2", "g2", "be2"]])

    key = "prog"
    if key not in _CACHE:
        _CACHE[key] = _build_program()
    nc = _CACHE[key]

    in_maps = []
    for b in range(B):
        m = dict(consts)
        m["z1"] = np.ascontiguousarray(z1[b])
        m["z2"] = np.ascontiguousarray(z2[b])
        in_maps.append(m)

    import os
    trace = bool(int(os.environ.get("KERNEL_TRACE", "0")))
    res = run_bass_kernel_spmd(nc, in_maps, list(range(B)), trace=trace)
    LAST_RESULT = res
    out = np.stack([res.results[b]["out"] for b in range(B)], axis=0)
    return out.astype(np.float32)
